# revision 39
# baseline (speedup 1.0000x reference)
"""Fused multi-head attention (QKV proj + RoPE + causal softmax + out proj)
for Trainium2, sharded over 8 NeuronCores.

Sharding: data-parallel over batch (B=2) x tensor-parallel over heads
(16 heads -> 4 per core).  Each core computes, for its (batch, head-group):
  qT/kT = wq/wk^T-projections in [d, s] layout (CDT matmuls, fp32 PSUM)
  RoPE applied on-chip (DVE pair-swap via stream_shuffle + mul/add)
  scoresT[kp, q] = krot^T.T @ qrot (one K=128 matmul per tile)
  causal masking via a PE-accumulated triangular constant on diagonal tiles
  expT = exp(scale * scoresT) on ACT, software-pipelined one kt-tile ahead
  PV with a ones-augmented V column => unnormalized out + softmax denominator
  normalize (DVE reciprocal + tensor_scalar), PE-transpose to attnT[d, s]
  partial output y_g = attnT.T @ wo_rows  (summed over head-groups on host)

Scheduling notes (v3):
  - weights (wq/wk/wv) are SBUF-resident, streamed once in 4-kc chunks; x is
    loaded once per s-block (4-kc chunks) and reused by all three projections.
  - the whole attention phase is ONE flat software pipeline over (qb, h, pr)
    pair-steps: scores/exp run L=2 steps ahead of PV, across head and q-block
    boundaries, so neither PE nor ACT drains at boundaries.
  - exp is trimmed to start at the diagonal on diagonal pairs (less ACT work
    exactly where the pipeline is ACT-tight).
  - per-head normalize runs inline; the PE transposes of head h and the
    out-projection tiles of q-block qb-1 are deferred "units" drip-fed into
    later steps, filling PE time while ACT catches up.
  - out-projection tiles stage into a [P, D] SBUF tile per s-tile and leave
    as ONE bf16 DMA per s-tile (y is bf16; host sums partials in fp64).

Inputs arrive full-size; host slices/transposes, feeds 8 SPMD cores, and
sums the 4 head-group partials per batch at the end.
"""

import math

import numpy as np

import concourse.bacc as bacc
import concourse.mybir as mybir
from concourse import tile
from concourse.bass_utils import run_bass_kernel_spmd

B, S, D, H = 2, 2048, 2048, 16
NCORES = 8
HG = 4  # heads per core
HD = D // H  # 128
DG = HG * HD  # 512 = per-core slice of D
P = 128
NKC = D // P  # 16 contraction chunks
SBLK = 512  # s-block width in projection passes
NSB = S // SBLK
NST = S // P  # 16 s-tiles of 128
QB = 512  # q-block width in attention
NQB = S // QB
NQT = QB // P  # q-subtiles per block
EB = 512  # e-block width in out-projection
NEB = D // EB

F32 = mybir.dt.float32
EXP = mybir.ActivationFunctionType.Exp
SCALE = 1.0 / math.sqrt(HD)
SWAP32 = [i ^ 1 for i in range(32)]
NEG = -1.0e9

COMPUTE_DTYPE = "bfloat16"


def build_program(variant: str, dump: bool = False, cdt_name: str | None = None):
    """variant: 'causal' | 'none' | 'general'"""
    CDT = getattr(mybir.dt, cdt_name or COMPUTE_DTYPE)
    nc = bacc.Bacc("TRN2", target_bir_lowering=False, debug=False)
    # host pre-swizzles x and the qkv weights partition-major so every DMA
    # chunk is >=4KB contiguous per partition (line-rate descriptors; the
    # row-major layouts fed 1KB segments and halved effective DMA rate)
    xT = nc.dram_tensor("xT", [P, NSB * NKC * SBLK], CDT, kind="ExternalInput")
    wq = nc.dram_tensor("wq", [P, NKC * DG], CDT, kind="ExternalInput")
    wk = nc.dram_tensor("wk", [P, NKC * DG], CDT, kind="ExternalInput")
    wv = nc.dram_tensor("wv", [P, NKC * DG], CDT, kind="ExternalInput")
    wo = nc.dram_tensor("wo", [DG, D], CDT, kind="ExternalInput")
    cosT = nc.dram_tensor("cosT", [HD, S], CDT, kind="ExternalInput")
    sinT = nc.dram_tensor("sinT", [HD, S], CDT, kind="ExternalInput")
    ident = nc.dram_tensor("ident", [P, P], CDT, kind="ExternalInput")
    tri = None
    maskT = None
    if variant == "causal":
        # 0/1 keep-mask in [kp, q] layout (1 where kp <= q): multiplied into
        # the exp'd diagonal tiles on the DVE (cheaper than a PE mask-matmul)
        tri = nc.dram_tensor("tri", [P, P], CDT, kind="ExternalInput")
    elif variant == "general":
        # mask.T pre-scaled by sqrt(HD) on host so exp's scale recovers it
        maskT = nc.dram_tensor("maskT", [S, S], CDT, kind="ExternalInput")
    y = nc.dram_tensor("y", [S, D], CDT, kind="ExternalOutput")
    d_qrot = d_krot = d_vaug = d_attnT = None
    if dump:
        d_qrot = nc.dram_tensor("d_qrot", [P, HG, S], CDT, kind="ExternalOutput")
        d_krot = nc.dram_tensor("d_krot", [P, HG, S], CDT, kind="ExternalOutput")
        d_vaug = nc.dram_tensor("d_vaug", [P, NST, HG, HD + 2], CDT, kind="ExternalOutput")
        d_attnT = nc.dram_tensor("d_attnT", [P, HG, S], CDT, kind="ExternalOutput")

    with tile.TileContext(nc) as tc:
        with (
            tc.tile_pool(name="const", bufs=1) as constp,
            tc.tile_pool(name="big", bufs=1) as bigp,
            # 6 PSUM banks cycled between proj accumulators / scores / PV-aug
            tc.tile_pool(name="ps", bufs=1, space="PSUM") as pspool,
            # 2 PSUM banks shared by transposes + out-projection
            tc.tile_pool(name="psaux", bufs=2, space="PSUM") as psaux,
        ):
            # --- constants on the vector DMA queue (not startup-critical) ---
            tid = constp.tile([P, P], CDT)
            nc.sync.dma_start(tid[:], ident[:])
            ttri = None
            if variant == "causal":
                ttri = constp.tile([P, P], CDT)
                nc.sync.dma_start(ttri[:], tri[:])

            # --- resident weights in kc chunks on the scalar queue, ordered by
            # first-use time: wq, wk, cos/sin (RoPE of the first q drain), wv.
            # Fine granularity lets the first matmul start ~1.5us after DMA
            # comes up instead of waiting on MB-sized chunks. ---
            wres = {}
            for pname in ("q", "k", "v"):
                wres[pname] = bigp.tile([P, NKC, DG], CDT, tag=f"w{pname}", name=f"w{pname}")

            def stream_w(pname, wdram, eng, fine=0):
                # partition-major contiguous chunks (line-rate descriptors);
                # the first `fine` kcs go as single-kc chunks so the very
                # first matmuls wait on 128 KB, not 512 KB
                kc = 0
                while kc < NKC:
                    n = 1 if kc < fine else 4
                    eng.dma_start(
                        wres[pname][:, kc : kc + n, :],
                        wdram[:, kc * DG : (kc + n) * DG].rearrange(
                            "p (c e) -> p c e", e=DG
                        ),
                    )
                    kc += n

            # sync HWDGE ring carries the weights in first-use order and ends
            # with wo (not needed until the attention phase); the scalar ring
            # carries x. During the DMA-saturated ramp only wq+x0 compete.
            stream_w("q", wq, nc.sync, fine=4)
            stream_w("k", wk, nc.sync)
            tcos = constp.tile([HD, S], CDT)
            nc.sync.dma_start(tcos[:], cosT[:])
            tsin = constp.tile([HD, S], CDT)
            nc.sync.dma_start(tsin[:], sinT[:])
            stream_w("v", wv, nc.sync)
            wo_sb = bigp.tile([P, HG, D], CDT, tag="wo")
            nc.sync.dma_start(
                wo_sb[:], wo.ap().rearrange("(dc p) e -> p dc e", p=P)
            )

            warm = constp.tile([P, 64], CDT, name="warm")
            nc.vector.memset(warm[:], 0.125)
            ps_w = pspool.tile([P, 64], F32, tag="aug", bufs=2, name="warmps")
            for _ in range(48):
                nc.tensor.matmul(
                    ps_w[0:64, :], warm[:, 0:64], warm[:], start=True, stop=True
                )

            qrot = bigp.tile([P, HG, S], CDT, tag="qrot")
            krot = bigp.tile([P, HG, S], CDT, tag="krot")
            vaug = bigp.tile([P, NST, HG, HD + 2], CDT, tag="vaug")
            ones_view = vaug[:, :, :, HD : HD + 2]
            attnT = bigp.tile([P, HG, S], CDT, tag="attnT")

            # PSUM plan (8 banks): 2x 'sc2' tiles of 2 banks (kt-pair scores,
            # exp'd by a single ACT instruction each), 2x 'aug' 1-bank tiles
            # holding two PV accumulators apiece, 2x 'tr' banks (out-proj).
            # Projection passes cycle the same 8 banks as 4 accumulators/pass.
            acc_idx = [0]

            def proj_accs(name):
                i = acc_idx[0]
                acc_idx[0] += 1
                if i % 2 == 0:
                    t0 = pspool.tile([P, 2, SBLK], F32, tag="sc2", bufs=2, name=f"{name}a")
                    t1 = pspool.tile([P, 2, SBLK], F32, tag="sc2", bufs=2, name=f"{name}b")
                    return [t0[:, 0, :], t0[:, 1, :], t1[:, 0, :], t1[:, 1, :]]
                a0 = pspool.tile([P, SBLK], F32, tag="aug", bufs=2, name=f"{name}a0")
                a1 = pspool.tile([P, SBLK], F32, tag="aug", bufs=2, name=f"{name}a1")
                r0 = psaux.tile([P, SBLK], F32, tag="tr", name=f"{name}r0")
                r1 = psaux.tile([P, SBLK], F32, tag="tr", name=f"{name}r1")
                return [a0, a1, r0, r1]

            # ---------------- projections + RoPE ----------------
            with (
                tc.tile_pool(name="xpool", bufs=2) as xpool,
                tc.tile_pool(name="rope", bufs=3) as ropep,
            ):
                for sb in range(NSB):
                    # x rows for this s-block, loaded once, reused by q/k/v.
                    # Rides the scalar HWDGE ring: no Q7 descriptor-gen
                    # serialization (the ramp is DMA-saturated).
                    xt = xpool.tile([P, NKC, SBLK], CDT, tag="xt", name="xt")
                    kc = 0
                    while kc < NKC:
                        n = 1 if (sb == 0 and kc < 4) else 4
                        base = (sb * NKC + kc) * SBLK
                        nc.scalar.dma_start(
                            xt[:, kc : kc + n, :],
                            xT[:, base : base + n * SBLK].rearrange(
                                "p (c s) -> p c s", s=SBLK
                            ),
                        )
                        kc += n
                    if sb == 0:
                        nc.vector.memset(ones_view, 1.0)
                        # pull the exp ACT-table load (~2.7us) off the
                        # attention critical path — but issued AFTER sb0's x
                        # dma_starts (they share the ACT instruction stream,
                        # and the table load was delaying the x feed by ~3us)
                        dmy = constp.tile([P, 1], CDT, name="dmy")
                        nc.scalar.activation(dmy[:], warm[:, 0:1], EXP, scale=1.0)
                    # last s-block runs (v, k, q): the k-pass holds the sc2
                    # PSUM ring the attention scores want first — running it
                    # one pass earlier frees sc2 before the phase handoff
                    order = ("q", "k", "v") if sb < NSB - 1 else ("v", "k", "q")
                    for proj in order:
                        wt = wres[proj]
                        nun = SBLK // P if proj == "v" else HG
                        pss = proj_accs(f"ps_{proj}_{sb}")
                        for kc in range(NKC):
                            if proj in ("q", "k"):
                                for dt in range(HG):
                                    nc.tensor.matmul(
                                        pss[dt][:],
                                        wt[:, kc, dt * HD : (dt + 1) * HD],
                                        xt[:, kc, :],
                                        start=(kc == 0),
                                        stop=(kc == NKC - 1),
                                    )
                            else:
                                for st in range(SBLK // P):
                                    nc.tensor.matmul(
                                        pss[st][:],
                                        xt[:, kc, st * P : (st + 1) * P],
                                        wt[:, kc, :],
                                        start=(kc == 0),
                                        stop=(kc == NKC - 1),
                                    )
                        if proj in ("q", "k"):
                            dstbuf = qrot if proj == "q" else krot
                            ssl = slice(sb * SBLK, (sb + 1) * SBLK)
                            # drain all four PSUM banks first (fast copies),
                            # then run the RoPE chains from SBUF
                            qsbs = []
                            for dt in range(HG):
                                qsb = ropep.tile(
                                    [P, SBLK], CDT, tag=f"qsb{dt}", name="qsb"
                                )
                                nc.scalar.copy(qsb[:], pss[dt][:])
                                qsbs.append(qsb)
                            for dt in range(HG):
                                qsb = qsbs[dt]
                                tsw = ropep.tile([P, SBLK], CDT, tag="tsw", name="tsw")
                                nc.vector.stream_shuffle(tsw[:], qsb[:], SWAP32)
                                t1 = ropep.tile([P, SBLK], CDT, tag="t1", name="t1")
                                nc.vector.tensor_mul(t1[:], qsb[:], tcos[:, ssl])
                                t2 = ropep.tile([P, SBLK], CDT, tag="t2", name="t2")
                                nc.vector.tensor_mul(t2[:], tsw[:], tsin[:, ssl])
                                nc.vector.tensor_add(dstbuf[:, dt, ssl], t1[:], t2[:])
                        else:
                            # v drains ride the DVE (ACT FIFO head-of-line
                            # blocking at sb seams), EXCEPT the last sb's
                            # aug-bank accumulators (st 0/1): those gate the
                            # attention pipeline's first PV, and ACT is idle
                            # right at the phase handoff.
                            for st in range(SBLK // P):
                                st_g = sb * (SBLK // P) + st
                                src = pss[st][:].rearrange("p (h d) -> p h d", d=HD)
                                if sb == NSB - 1 and st < 2:
                                    nc.scalar.copy(vaug[:, st_g, :, 0:HD], src)
                                else:
                                    nc.vector.tensor_copy(vaug[:, st_g, :, 0:HD], src)

            if dump:
                nc.sync.dma_start(d_qrot.ap(), qrot[:])
                nc.sync.dma_start(d_krot.ap(), krot[:])
                nc.sync.dma_start(d_vaug.ap(), vaug[:])

            # ---------------- attention + interleaved out-projection ----------------
            with (
                tc.tile_pool(name="mask", bufs=2) as maskp,
                tc.tile_pool(name="expp", bufs=4) as epool,
                tc.tile_pool(name="small", bufs=4) as smallp,
                tc.tile_pool(name="normp", bufs=1) as npool,
                tc.tile_pool(name="outp", bufs=2) as outp,
            ):
                # deferred PE work (transposes of the previous head,
                # out-proj tiles of the previous q-block), drip-fed into
                # the pipeline so the PE stays ahead of the ACT engine
                units = []
                pending_out = []

                def emit_units(k):
                    for _ in range(min(k, len(units))):
                        units.pop(0)()

                # output staging: 4 eb tiles of one s-tile collect in a
                # [P, D] bf16 tile, then leave as a single 512 KB DMA
                ostage = {}

                def outproj_unit(st, eb, on_act=False):
                    def run():
                        ps_o = psaux.tile([P, EB], F32, tag="tr", name=f"o{st}_{eb}")
                        for dc in range(HG):
                            nc.tensor.matmul(
                                ps_o[:],
                                attnT[:, dc, st * P : (st + 1) * P],
                                wo_sb[:, dc, eb * EB : (eb + 1) * EB],
                                start=(dc == 0),
                                stop=(dc == HG - 1),
                            )
                        if eb == 0:
                            ostage[st] = outp.tile(
                                [P, NEB, EB], CDT, tag="outsb", name="outsb"
                            )
                        stg = ostage[st]
                        if on_act:
                            nc.scalar.copy(stg[:, eb, :], ps_o[:])
                        else:
                            nc.vector.tensor_copy(stg[:, eb, :], ps_o[:])
                        if eb % 2 == 1:
                            # half-row DMAs (256 KB): earlier drain start and
                            # a shorter last-DMA tail
                            half = eb // 2
                            nc.sync.dma_start(
                                y[
                                    st * P : (st + 1) * P,
                                    half * (D // 2) : (half + 1) * (D // 2),
                                ],
                                stg[:, 2 * half : 2 * half + 2, :].rearrange(
                                    "p c e -> p (c e)"
                                ),
                            )
                    return run

                def transpose_unit(h, qt_g, attn_n):
                    def run():
                        ps_t = psaux.tile([P, P], CDT, tag="tr", name="tr")
                        nc.tensor.transpose(ps_t[:], attn_n[:], tid[:])
                        nc.vector.tensor_copy(
                            attnT[:, h, qt_g * P : (qt_g + 1) * P], ps_t[:]
                        )
                    return run

                def finish_qt(h, qb, qt, aug):
                    """normalize one finished q-subtile accumulator straight
                    out of PSUM (DVE); defer the PE transpose as a unit."""
                    qt_g = qb * NQT + qt
                    rec = smallp.tile([P, 1], F32, tag="rec", name="rec", bufs=8)
                    nc.vector.reciprocal(rec[:], aug[:, HD : HD + 1])
                    attn_n = npool.tile(
                        [P, HD], CDT, tag="attn_n", name="attn_n", bufs=16
                    )
                    nc.vector.tensor_scalar_mul(attn_n[:], aug[:, 0:HD], rec[:])
                    units.append(transpose_unit(h, qt_g, attn_n))
                    # once the last head finished this s-tile, its out-proj
                    # can go. Deferred one finish behind the transpose (via
                    # pending_out) so the transpose's PSUM slot is long
                    # released before the next out-proj allocation wants it.
                    if h == HG - 1:
                        units.extend(pending_out)
                        pending_out.clear()
                        for eb in range(NEB):
                            pending_out.append(
                                outproj_unit(
                                    qt_g,
                                    eb,
                                    on_act=(qb == NQB - 1 and eb % 2 == 0),
                                )
                            )

                # flat (qb, h, pr) step list: the whole attention phase is one
                # software pipeline, scores/exp L steps ahead of PV, across
                # head/q-block boundaries
                step_list = []
                for qb in range(NQB):
                    nkt = NQT * (qb + 1) if variant == "causal" else NST
                    for h in range(HG):
                        for pr in range(nkt // 2):
                            step_list.append((qb, h, pr))

                state = {}
                mts_by_qb = {}

                def scores_step(qb, h, pr):
                    st = state.setdefault((qb, h), {"augp": [None, None], "exps": {}})
                    ps2 = pspool.tile(
                        [P, 2, QB], F32, tag="sc2", bufs=2, name="scores"
                    )
                    for i in (0, 1):
                        kt = 2 * pr + i
                        j = kt - NQT * qb  # diag index (causal)
                        if variant == "causal" and j >= 0:
                            nc.tensor.matmul(
                                ps2[:, i, j * P : QB],
                                krot[:, h, kt * P : (kt + 1) * P],
                                qrot[:, h, qb * QB + j * P : (qb + 1) * QB],
                                start=True,
                                stop=True,
                            )
                        else:
                            last = variant != "general"
                            nc.tensor.matmul(
                                ps2[:, i, :],
                                krot[:, h, kt * P : (kt + 1) * P],
                                qrot[:, h, qb * QB : (qb + 1) * QB],
                                start=True,
                                stop=last,
                            )
                            if variant == "general":
                                nc.tensor.matmul(
                                    ps2[:, i, :],
                                    tid[:],
                                    mts_by_qb[qb][:, kt, :],
                                    start=False,
                                    stop=True,
                                )
                    # one ACT instruction covers both kt strips, starting at
                    # the diagonal (earlier cols of diagonal pairs are never
                    # read by PV, so skip exp-ing them)
                    j0 = max(0, 2 * pr - NQT * qb) if variant == "causal" else 0
                    texp = epool.tile([P, 2, QB], CDT, tag="exp", name="exp")
                    nc.scalar.activation(
                        texp[:, :, j0 * P :], ps2[:, :, j0 * P :], EXP, scale=SCALE
                    )
                    if variant == "causal":
                        # zero the below-diagonal part of each diagonal tile
                        # (DVE 0/1 mask-mul; replaces a PE mask-matmul)
                        for i in (0, 1):
                            j = 2 * pr + i - NQT * qb
                            if 0 <= j < NQT:
                                nc.vector.tensor_mul(
                                    texp[:, i, j * P : (j + 1) * P],
                                    texp[:, i, j * P : (j + 1) * P],
                                    ttri[:],
                                )
                    st["exps"][pr] = texp

                def pv_step(qb, h, pr):
                    st = state[(qb, h)]
                    texp = st["exps"].pop(pr)
                    augp = st["augp"]
                    for i in (0, 1):
                        kt = 2 * pr + i
                        j = kt - NQT * qb
                        for qt in range(NQT):
                            if variant == "causal" and (
                                j > qt or kt > NQT * qb + qt
                            ):
                                continue
                            if kt == 0 and qt % 2 == 0:
                                augp[qt // 2] = pspool.tile(
                                    [P, 2, 256],
                                    F32,
                                    tag="aug",
                                    bufs=2,
                                    name=f"aug{qt // 2}",
                                )
                            aug = augp[qt // 2][:, qt % 2, :]
                            last_kt = (
                                NQT * qb + qt if variant == "causal" else NST - 1
                            )
                            nc.tensor.matmul(
                                aug[:, 0 : HD + 2],
                                texp[:, i, qt * P : (qt + 1) * P],
                                vaug[:, kt, h, :],
                                start=(kt == 0 and qt % 2 == 0),
                                stop=(kt == last_kt),
                            )
                            if kt == last_kt:
                                finish_qt(h, qb, qt, aug)

                # scores/exp run 3 pair-steps ahead of PV: exp latency (and
                # the ACT drain queue at the phase handoff) never gates PV.
                # texp ring: 4 tiles in flight (t-3..t) -> epool bufs=4 holds.
                LEAD = 3
                nstep = len(step_list)
                cur_qb = -1
                for t in range(nstep + LEAD):
                    if t < nstep:
                        qb, h, pr = step_list[t]
                        if qb != cur_qb:
                            cur_qb = qb
                            if variant == "general":
                                mts = maskp.tile(
                                    [P, NST, QB], CDT, tag="mt", name="mt"
                                )
                                nc.sync.dma_start(
                                    mts[:],
                                    maskT[:, qb * QB : (qb + 1) * QB].rearrange(
                                        "(kt p) q -> p kt q", p=P
                                    ),
                                )
                                mts_by_qb[qb] = mts
                        scores_step(qb, h, pr)
                    emit_units(2)
                    if t >= LEAD:
                        pv_step(*step_list[t - LEAD])

                # flush: remaining transposes + final q-block's out-projection
                units.extend(pending_out)
                pending_out.clear()
                emit_units(len(units))

                if dump:
                    nc.sync.dma_start(d_attnT.ap(), attnT[:])

    nc.compile()
    return nc


_PROGRAM_CACHE: dict[str, object] = {}
_last_in_maps = None


def _get_program(variant: str):
    key = f"{variant}:{COMPUTE_DTYPE}"
    if key not in _PROGRAM_CACHE:
        _PROGRAM_CACHE[key] = build_program(variant)
    return _PROGRAM_CACHE[key]


def _detect_variant(mask: np.ndarray) -> str:
    if not np.any(mask):
        return "none"
    causal = np.triu(np.full((S, S), NEG, dtype=np.float32), 1)
    if np.array_equal(mask, causal):
        return "causal"
    return "general"


def _np_cdt():
    if COMPUTE_DTYPE == "bfloat16":
        import ml_dtypes

        return ml_dtypes.bfloat16
    return np.float32


def make_in_maps(x, wq, wk, wv, wo, cos, sin, mask, variant):
    npdt = _np_cdt()
    cosT = np.repeat(cos.T, 2, axis=0)  # [HD, S]
    sinT = np.repeat(sin.T, 2, axis=0)
    sinT = sinT.copy()
    sinT[0::2, :] *= -1.0  # row 2i holds -sin, row 2i+1 holds +sin
    shared = {
        "cosT": np.ascontiguousarray(cosT).astype(npdt),
        "sinT": np.ascontiguousarray(sinT).astype(npdt),
        "ident": np.eye(P, dtype=np.float32).astype(npdt),
    }
    if variant == "causal":
        # scoresT layout is [kp, q]: keep kp <= q -> upper triangle incl. diag
        shared["tri"] = np.triu(np.ones((P, P), dtype=np.float32), 0).astype(npdt)
    elif variant == "general":
        shared["maskT"] = np.ascontiguousarray(mask.T * math.sqrt(HD)).astype(npdt)

    def prep_x(xb):
        # [S, D] -> partition-major [P, NSB*NKC*SBLK]: entry (p, sb, kc, s)
        # = xT[kc*P + p, sb*SBLK + s]
        xt = xb.T.reshape(NKC, P, NSB, SBLK).transpose(1, 2, 0, 3)
        return np.ascontiguousarray(xt.reshape(P, NSB * NKC * SBLK)).astype(npdt)

    def prep_w(w):
        # [D, DG] -> partition-major [P, NKC*DG]: entry (p, kc, e)
        # = w[kc*P + p, e]
        wp = w.reshape(NKC, P, DG).transpose(1, 0, 2)
        return np.ascontiguousarray(wp.reshape(P, NKC * DG)).astype(npdt)

    xTs = [prep_x(x[b]) for b in range(B)]
    in_maps = []
    for core in range(NCORES):
        b, g = divmod(core, NCORES // B)
        sl = slice(g * DG, (g + 1) * DG)
        in_maps.append(
            {
                "xT": xTs[b],
                "wq": prep_w(wq[:, sl]),
                "wk": prep_w(wk[:, sl]),
                "wv": prep_w(wv[:, sl]),
                "wo": np.ascontiguousarray(wo[sl, :]).astype(npdt),
                **shared,
            }
        )
    return in_maps


def kernel(x, wq, wk, wv, wo, cos, sin, mask):
    x = np.asarray(x, dtype=np.float32)
    wq = np.asarray(wq, dtype=np.float32)
    wk = np.asarray(wk, dtype=np.float32)
    wv = np.asarray(wv, dtype=np.float32)
    wo = np.asarray(wo, dtype=np.float32)
    cos = np.asarray(cos, dtype=np.float32)
    sin = np.asarray(sin, dtype=np.float32)
    mask = np.asarray(mask, dtype=np.float32)

    variant = _detect_variant(mask)
    nc = _get_program(variant)
    in_maps = make_in_maps(x, wq, wk, wv, wo, cos, sin, mask, variant)

    global _last_in_maps
    _last_in_maps = in_maps

    res = run_bass_kernel_spmd(nc, in_maps, core_ids=list(range(NCORES)))

    out = np.empty((B, S, D), dtype=np.float32)
    gpb = NCORES // B
    for b in range(B):
        acc = np.zeros((S, D), dtype=np.float64)
        for g in range(gpb):
            acc += res.results[b * gpb + g]["y"].astype(np.float64)
        out[b] = acc.astype(np.float32)
    return out

